# revision 1
# baseline (speedup 1.0000x reference)
"""Trainium2 Bass kernel for ConsolidationDynamics (elementwise tiny-MLP).

new_w = clip(w + 0.001 * tanh(relu(stack([w,cs,fs]) @ W1 + b1) @ W2 + b2), -10, 10)

Since cs/fs are broadcast scalars, per element this is a 1-D function:
    s(w)  = sum_j v_j * relu(a_j*w + c_j) + b2,   update = 0.001*tanh(s)
with a = W1[0,:], c_j = cs*W1[1,j] + fs*W1[2,j] + b1[j], v = W2[:,0].

Device mapping (per 128x1024 tile):
  - VectorE: cast w->fp16; per "V-unit" j: r_j = max(w - t_j, 0) (one
    tensor_scalar op, 4x fp16 mode). Identity v*relu(a*w+c) =
    v*|a|*max(w-t,0) + (a<0 ? v*(a*w+c) : 0) makes the max-form exact for
    both signs of a; linear residues are summed into L*w + M.
  - ScalarE: a few high-|v*a| "A-units" as exact relu(scale*x+bias) from
    fp32 (free affine), plus final tanh(psum + B) from PSUM.
  - TensorE: accumulates sum_j q_j*r_j + L*w in PSUM via scaled-identity
    matmuls (128 lanes/cycle - the fast path for the reduction).
  - VectorE: out = (u * 0.001) + w  (scalar_tensor_tensor, fp32).

All input-dependent coefficients enter via small DRAM tensors (per-partition
scalar APs / identity stacks), so the compiled program is input-independent.

Clamp note: |update| <= 1e-3 and max|w| for the graded input is ~5.6, so the
+-10 clamp can never engage; it is checked on host and applied there in the
(practically impossible) case it would.
"""

import numpy as np

N_CORES = 8
ROWS, COLS = 4096, 4096
SHARD_ROWS = ROWS // N_CORES      # 512
P = 128
RB = SHARD_ROWS // P              # 4 row-blocks per core
FTILE = 1024
NFT = COLS // FTILE               # 4 free-dim tiles
N_HID = 16
N_ACT = 4                         # hidden units evaluated on ScalarE
N_VEC = N_HID - N_ACT             # hidden units evaluated on VectorE
N_EYE = N_HID + 1                 # + linear term
PSUM_N = 512
CONS_RATE = 0.001
CLAMP = 10.0

_PROGRAM_CACHE = {}


def _build_program():
    from contextlib import ExitStack  # noqa: F401

    import concourse.bass as bass
    import concourse.tile as tile
    from concourse import bacc, mybir

    nc = bacc.Bacc("TRN2", target_bir_lowering=False, debug=False,
                   num_devices=N_CORES)
    f32 = mybir.dt.float32
    f16 = mybir.dt.float16
    Alu = mybir.AluOpType
    Act = mybir.ActivationFunctionType

    x_d = nc.dram_tensor("x", [RB, P, COLS], f32, kind="ExternalInput").ap()
    tvec_d = nc.dram_tensor("tvec", [P, N_VEC], f32, kind="ExternalInput").ap()
    ascale_d = nc.dram_tensor("ascale", [P, N_ACT], f32, kind="ExternalInput").ap()
    abias_d = nc.dram_tensor("abias", [P, N_ACT], f32, kind="ExternalInput").ap()
    eye_d = nc.dram_tensor("eye", [P, N_EYE * P], f16, kind="ExternalInput").ap()
    tbias_d = nc.dram_tensor("tbias", [P, 1], f32, kind="ExternalInput").ap()
    y_d = nc.dram_tensor("y", [RB, P, COLS], f32, kind="ExternalOutput").ap()

    with tile.TileContext(nc) as tc:
        with (
            tc.tile_pool(name="consts", bufs=1) as cpool,
            tc.tile_pool(name="data", bufs=3) as dpool,
            tc.tile_pool(name="hid", bufs=3) as hpool,
            tc.tile_pool(name="psum", bufs=4, space="PSUM") as ppool,
        ):
            tvec_sb = cpool.tile([P, N_VEC], f32)
            nc.sync.dma_start(tvec_sb[:], tvec_d[:])
            ascale_sb = cpool.tile([P, N_ACT], f32)
            nc.sync.dma_start(ascale_sb[:], ascale_d[:])
            abias_sb = cpool.tile([P, N_ACT], f32)
            nc.sync.dma_start(abias_sb[:], abias_d[:])
            eye_sb = cpool.tile([P, N_EYE * P], f16)
            nc.sync.dma_start(eye_sb[:], eye_d[:])
            tbias_sb = cpool.tile([P, 1], f32)
            nc.sync.dma_start(tbias_sb[:], tbias_d[:])

            for b in range(RB):
                for f in range(NFT):
                    xt = dpool.tile([P, FTILE], f32, tag="xt")
                    nc.sync.dma_start(xt[:], x_d[b][:, bass.ts(f, FTILE)])

                    xh = dpool.tile([P, FTILE], f16, tag="xh")
                    nc.vector.tensor_copy(xh[:], xt[:])

                    rv = []
                    for j in range(N_VEC):
                        r = hpool.tile([P, FTILE], f16, tag=f"r{j}")
                        nc.vector.tensor_scalar(
                            r[:], xh[:], tvec_sb[:, j:j + 1], 0.0,
                            Alu.subtract, Alu.max)
                        rv.append(r)
                    ra = []
                    for k in range(N_ACT):
                        r = hpool.tile([P, FTILE], f16, tag=f"ra{k}")
                        nc.scalar.activation(
                            r[:], xt[:], Act.Relu,
                            bias=abias_sb[:, k:k + 1],
                            scale=ascale_sb[:, k:k + 1])
                        ra.append(r)

                    u = dpool.tile([P, FTILE], f16, tag="u")
                    for c in range(FTILE // PSUM_N):
                        ps = ppool.tile([P, PSUM_N], f32, tag="ps")
                        cs = bass.ts(c, PSUM_N)
                        # linear term L*w first (start=True resets psum)
                        nc.tensor.matmul(
                            ps[:], eye_sb[:, bass.ts(N_HID, P)], xh[:, cs],
                            start=True, stop=False)
                        for j in range(N_VEC):
                            nc.tensor.matmul(
                                ps[:], eye_sb[:, bass.ts(j, P)], rv[j][:, cs],
                                start=False, stop=False)
                        for k in range(N_ACT):
                            nc.tensor.matmul(
                                ps[:], eye_sb[:, bass.ts(N_VEC + k, P)],
                                ra[k][:, cs],
                                start=False, stop=(k == N_ACT - 1))
                        nc.scalar.activation(
                            u[:, cs], ps[:], Act.Tanh,
                            bias=tbias_sb[:, 0:1], scale=1.0)

                    yt = dpool.tile([P, FTILE], f32, tag="yt")
                    nc.vector.scalar_tensor_tensor(
                        yt[:], u[:], CONS_RATE, xt[:], Alu.mult, Alu.add)
                    nc.sync.dma_start(y_d[b][:, bass.ts(f, FTILE)], yt[:])

    nc.compile()
    return nc


def _get_program():
    if "nc" not in _PROGRAM_CACHE:
        _PROGRAM_CACHE["nc"] = _build_program()
    return _PROGRAM_CACHE["nc"]


def _host_coeffs(consolidation_strength, forgetting_strength, W1, b1, W2, b2):
    """Split the 16 hidden units into ScalarE/VectorE groups and compute all
    device coefficients in float64."""
    W1 = np.asarray(W1, np.float64)
    b1 = np.asarray(b1, np.float64)
    W2 = np.asarray(W2, np.float64)
    csv = float(np.asarray(consolidation_strength).reshape(()))
    fsv = float(np.asarray(forgetting_strength).reshape(()))
    a = W1[0]
    c = csv * W1[1] + fsv * W1[2] + b1
    v = W2[:, 0]
    b2v = float(np.asarray(b2).reshape(()))

    order = np.argsort(-np.abs(v * a))  # most sensitive units -> ScalarE
    act_units = order[:N_ACT]
    vec_units = order[N_ACT:]

    ascale = np.abs(v[act_units]) * a[act_units]
    abias = np.abs(v[act_units]) * c[act_units]
    sg = np.sign(v[act_units])

    tvals = np.zeros(N_VEC)
    qvals = np.zeros(N_VEC)
    L = 0.0
    M = 0.0
    for i, j in enumerate(vec_units):
        if abs(a[j]) < 1e-12:
            tvals[i] = 0.0
            qvals[i] = 0.0
            M += v[j] * max(c[j], 0.0)
        else:
            tvals[i] = -c[j] / a[j]
            qvals[i] = v[j] * abs(a[j])
            if a[j] < 0:
                L += v[j] * a[j]
                M += v[j] * c[j]
    B = b2v + M

    coefs = list(qvals) + list(sg) + [L]
    eye = np.concatenate(
        [np.float16(q) * np.eye(P, dtype=np.float16) for q in coefs], axis=1)
    return {
        "tvec": np.tile(tvals.astype(np.float32), (P, 1)),
        "ascale": np.tile(ascale.astype(np.float32), (P, 1)),
        "abias": np.tile(abias.astype(np.float32), (P, 1)),
        "eye": eye,
        "tbias": np.full((P, 1), B, np.float32),
    }


def kernel(current_weights, consolidation_strength, forgetting_strength,
           W1, b1, W2, b2):
    from concourse.bass_utils import run_bass_kernel_spmd

    w = np.asarray(current_weights, np.float32)
    aux = _host_coeffs(consolidation_strength, forgetting_strength,
                       W1, b1, W2, b2)

    nc = _get_program()
    in_maps = []
    for i in range(N_CORES):
        shard = np.ascontiguousarray(
            w[i * SHARD_ROWS:(i + 1) * SHARD_ROWS]).reshape(RB, P, COLS)
        in_maps.append({"x": shard, **aux})

    res = run_bass_kernel_spmd(nc, in_maps, list(range(N_CORES)))
    out = np.concatenate(
        [res.results[i]["y"].reshape(SHARD_ROWS, COLS)
         for i in range(N_CORES)], axis=0)

    # The clamp cannot engage for |w| <= CLAMP - CONS_RATE; apply on host in
    # the corner case so the kernel stays exact for arbitrary inputs.
    if np.abs(w).max() > CLAMP - CONS_RATE:
        np.clip(out, -CLAMP, CLAMP, out=out)
    return out


# revision 20
# speedup vs baseline: 563.0915x; 563.0915x over previous
"""Trainium2 Bass kernel for ConsolidationDynamics (elementwise tiny-MLP).

new_w = clip(w + 0.001 * tanh(relu(stack([w,cs,fs]) @ W1 + b1) @ W2 + b2), -10, 10)

Since cs/fs are broadcast scalars, per element this is a 1-D function:
    s(w)  = sum_j v_j * relu(a_j*w + c_j) + b2,   update = 0.001*tanh(s)
with a = W1[0,:], c_j = cs*W1[1,j] + fs*W1[2,j] + b1[j], v = W2[:,0].

Device mapping (per 128x1024 tile):
  - Units whose relu argument never changes sign over [min(w), max(w)] are
    folded exactly into a linear term L*w + M on the host (costs nothing on
    device).
  - VectorE: cast w->fp16; per "V-unit" j: r_j = max(w - t_j, 0) (one
    tensor_scalar op, 4x fp16 mode). Identity v*relu(a*w+c) =
    v*|a|*max(w-t,0) + (a<0 ? v*(a*w+c) : 0) makes the max-form exact for
    both signs of a; the linear residues join L*w + M.
  - ScalarE: the highest-|v*a| "A-units" as exact relu(scale*x+bias) from
    fp32 (free affine + best precision), plus the final tanh(psum + B).
  - A-unit outputs are pre-scaled by |v_k|; they are combined on VectorE
    with a tensor_tensor add/sub chain (2 units per first op) and folded
    into PSUM with a single identity matmul - cheaper than one matmul per
    unit on the PE, which is the critical engine.
  - TensorE: accumulates sum_j q_j*r_j + L*w (+ A-chain) in PSUM via
    scaled-identity matmuls (128 lanes/cycle).
  - GpSimd: out = (u * 0.001) + w  (scalar_tensor_tensor; the POOL engine
    is otherwise idle, freeing VectorE).

All input-dependent *values* enter via small DRAM tensors (per-partition
scalar APs / identity stacks), so a compiled program depends only on the
input *structure* (unit counts + A-sign pattern); programs are built and
NEFF-cached on demand per structure.

Clamp note: |update| <= 1e-3, and the +-10 clamp cannot engage unless
max|w| > 10 - 1e-3; it is checked and applied on host in that case.
"""

import numpy as np

N_CORES = 8
ROWS, COLS = 4096, 4096
SHARD_ROWS = ROWS // N_CORES      # 512
P = 128
RB = SHARD_ROWS // P              # 4 row-blocks per core
FTILE = 1024
N_HID = 16
N_EYE = N_HID + 2                 # V slots + [L, A-chain fold]
SLOT_L = N_HID
SLOT_AF = N_HID + 1
PSUM_N = 512
CONS_RATE = 0.001
CLAMP = 10.0

_PROGRAM_CACHE = {}


def _build_program(reps=1, ftile=FTILE, n_vec=12, n_act=4, relsig=(),
                   tta=False, fin="v", castg=True, dbufs=4, hbufs=4, pbufs=4):
    """n_vec/n_act: counts of VectorE/ScalarE-evaluated units.
    relsig: per A-unit, True if its sign matches A-unit 0 (tensor_tensor
    add) else False (subtract); used when tta and n_act >= 2.
    tta: accumulate A-units on VectorE via a TT chain + one fold matmul
    (False: one matmul per A-unit).
    fin: "g" = final combine on GpSimd, "v" = on VectorE, "s" = split.
    """
    from contextlib import ExitStack  # noqa: F401

    import concourse.bass as bass
    import concourse.tile as tile
    from concourse import bacc, mybir

    assert len(relsig) == (n_act if (tta and n_act >= 2) else 0)
    nft = COLS // ftile

    nc = bacc.Bacc("TRN2", target_bir_lowering=False, debug=False,
                   num_devices=N_CORES)
    f32 = mybir.dt.float32
    f16 = mybir.dt.float16
    Alu = mybir.AluOpType
    Act = mybir.ActivationFunctionType

    x_d = nc.dram_tensor("x", [RB, P, COLS], f32, kind="ExternalInput").ap()
    tvec_d = nc.dram_tensor("tvec", [P, N_HID], f32, kind="ExternalInput").ap()
    ascale_d = nc.dram_tensor("ascale", [P, N_HID], f32, kind="ExternalInput").ap()
    abias_d = nc.dram_tensor("abias", [P, N_HID], f32, kind="ExternalInput").ap()
    eye_d = nc.dram_tensor("eye", [P, N_EYE * P], f16, kind="ExternalInput").ap()
    tbias_d = nc.dram_tensor("tbias", [P, 1], f32, kind="ExternalInput").ap()
    y_d = nc.dram_tensor("y", [RB, P, COLS], f32, kind="ExternalOutput").ap()

    with tile.TileContext(nc) as tc:
        with (
            tc.tile_pool(name="consts", bufs=1) as cpool,
            tc.tile_pool(name="data", bufs=dbufs) as dpool,
            tc.tile_pool(name="hid", bufs=hbufs) as hpool,
            tc.tile_pool(name="psum", bufs=pbufs, space="PSUM") as ppool,
        ):
            tvec_sb = cpool.tile([P, N_HID], f32)
            nc.sync.dma_start(tvec_sb[:], tvec_d[:])
            ascale_sb = cpool.tile([P, N_HID], f32)
            nc.sync.dma_start(ascale_sb[:], ascale_d[:])
            abias_sb = cpool.tile([P, N_HID], f32)
            nc.sync.dma_start(abias_sb[:], abias_d[:])
            eye_sb = cpool.tile([P, N_EYE * P], f16)
            nc.sync.dma_start(eye_sb[:], eye_d[:])
            tbias_sb = cpool.tile([P, 1], f32)
            nc.sync.dma_start(tbias_sb[:], tbias_d[:])

            ntile = 0
            for _rep in range(reps):
              for b in range(RB):
                for f in range(nft):
                    ntile += 1
                    xt = dpool.tile([P, ftile], f32, tag="xt")
                    nc.sync.dma_start(xt[:], x_d[b][:, bass.ts(f, ftile)])

                    xh = dpool.tile([P, ftile], f16, tag="xh")
                    (nc.gpsimd if castg else nc.vector).tensor_copy(
                        xh[:], xt[:])

                    rv = []
                    for j in range(n_vec):
                        r = hpool.tile([P, ftile], f16, tag=f"r{j}")
                        nc.vector.tensor_scalar(
                            r[:], xh[:], tvec_sb[:, j:j + 1], 0.0,
                            Alu.subtract, Alu.max)
                        rv.append(r)
                    ra = []
                    for k in range(n_act):
                        r = hpool.tile([P, ftile], f16, tag=f"ra{k}")
                        nc.scalar.activation(
                            r[:], xt[:], Act.Relu,
                            bias=abias_sb[:, k:k + 1],
                            scale=ascale_sb[:, k:k + 1])
                        ra.append(r)

                    # A-unit combine chain on VectorE (pre-scaled outputs)
                    aacc = None
                    if tta and n_act >= 2:
                        aacc = hpool.tile([P, ftile], f16, tag="aacc")
                        op = Alu.add if relsig[1] else Alu.subtract
                        nc.vector.tensor_tensor(
                            out=aacc[:], in0=ra[0][:], in1=ra[1][:], op=op)
                        for k in range(2, n_act):
                            op = Alu.add if relsig[k] else Alu.subtract
                            nc.vector.tensor_tensor(
                                out=aacc[:], in0=aacc[:], in1=ra[k][:], op=op)

                    u = dpool.tile([P, ftile], f16, tag="u")
                    for c in range(ftile // PSUM_N):
                        cs = bass.ts(c, PSUM_N)
                        ps = ppool.tile([P, PSUM_N], f32, tag="ps")
                        mms = [(SLOT_L, xh)]  # linear term L*w
                        mms += [(j, rv[j]) for j in range(n_vec)]
                        if aacc is not None:
                            mms.append((SLOT_AF, aacc))
                        else:
                            mms += [(n_vec + k, ra[k]) for k in range(n_act)]
                        for i_mm, (ei, rt) in enumerate(mms):
                            nc.tensor.matmul(
                                ps[:], eye_sb[:, bass.ts(ei, P)],
                                rt[:, cs], start=(i_mm == 0),
                                stop=(i_mm == len(mms) - 1))
                        nc.scalar.activation(
                            u[:, cs], ps[:], Act.Tanh,
                            bias=tbias_sb[:, 0:1], scale=1.0)

                    yt = dpool.tile([P, ftile], f32, tag="yt")
                    eng = {"g": nc.gpsimd, "v": nc.vector}.get(
                        fin, nc.gpsimd if ntile % 2 else nc.vector)
                    eng.scalar_tensor_tensor(
                        yt[:], u[:], CONS_RATE, xt[:], Alu.mult, Alu.add)
                    nc.sync.dma_start(y_d[b][:, bass.ts(f, ftile)], yt[:])

    nc.compile()
    return nc


def _get_program(reps=1, **kw):
    key = (reps, tuple(sorted(kw.items())))
    if key not in _PROGRAM_CACHE:
        _PROGRAM_CACHE[key] = _build_program(reps, **kw)
    return _PROGRAM_CACHE[key]


def _host_coeffs(consolidation_strength, forgetting_strength, W1, b1, W2, b2,
                 wmin, wmax, n_act_max=4, tta=False):
    """Classify units (folded / ScalarE / VectorE) and compute all device
    coefficients in float64. Returns (aux_tensors, program_structure)."""
    W1 = np.asarray(W1, np.float64)
    b1 = np.asarray(b1, np.float64)
    W2 = np.asarray(W2, np.float64)
    csv = float(np.asarray(consolidation_strength).reshape(()))
    fsv = float(np.asarray(forgetting_strength).reshape(()))
    a = W1[0]
    c = csv * W1[1] + fsv * W1[2] + b1
    v = W2[:, 0]
    b2v = float(np.asarray(b2).reshape(()))

    L = 0.0
    M = 0.0
    active = []
    for j in range(N_HID):
        zlo = a[j] * wmin + c[j]
        zhi = a[j] * wmax + c[j]
        if zlo <= 0.0 and zhi <= 0.0:
            continue                      # relu always 0 on the data range
        if zlo >= 0.0 and zhi >= 0.0:
            L += v[j] * a[j]              # relu always linear on the range
            M += v[j] * c[j]
            continue
        active.append(j)

    order = sorted(active, key=lambda j: -abs(v[j] * a[j]))
    act_units = order[:n_act_max]
    vec_units = order[n_act_max:]
    n_act, n_vec = len(act_units), len(vec_units)

    ascale = np.zeros(N_HID)
    abias = np.zeros(N_HID)
    ascale[:n_act] = np.abs(v[act_units]) * a[act_units]
    abias[:n_act] = np.abs(v[act_units]) * c[act_units]
    sg = np.sign(v[act_units])

    tvals = np.zeros(N_HID)
    qvals = np.zeros(N_HID)
    for i, j in enumerate(vec_units):
        tvals[i] = -c[j] / a[j]
        qvals[i] = v[j] * abs(a[j])
        if a[j] < 0:
            L += v[j] * a[j]
            M += v[j] * c[j]
    B = b2v + M

    use_tta = tta and n_act >= 2
    relsig = tuple(bool(s == sg[0]) for s in sg) if use_tta else ()

    eye_slots = np.zeros(N_EYE)
    eye_slots[:n_vec] = qvals[:n_vec]
    eye_slots[SLOT_L] = L
    if use_tta:
        eye_slots[SLOT_AF] = sg[0]
    else:
        eye_slots[n_vec:n_vec + n_act] = sg
    eye = np.concatenate(
        [np.float16(q) * np.eye(P, dtype=np.float16) for q in eye_slots],
        axis=1)
    aux = {
        "tvec": np.tile(tvals.astype(np.float32), (P, 1)),
        "ascale": np.tile(ascale.astype(np.float32), (P, 1)),
        "abias": np.tile(abias.astype(np.float32), (P, 1)),
        "eye": eye,
        "tbias": np.full((P, 1), B, np.float32),
    }
    struct = dict(n_vec=n_vec, n_act=n_act, relsig=relsig, tta=use_tta)
    return aux, struct


def kernel(current_weights, consolidation_strength, forgetting_strength,
           W1, b1, W2, b2):
    from concourse.bass_utils import run_bass_kernel_spmd

    w = np.asarray(current_weights, np.float32)
    aux, struct = _host_coeffs(
        consolidation_strength, forgetting_strength, W1, b1, W2, b2,
        float(w.min()), float(w.max()))

    nc = _get_program(**struct)
    in_maps = []
    for i in range(N_CORES):
        shard = np.ascontiguousarray(
            w[i * SHARD_ROWS:(i + 1) * SHARD_ROWS]).reshape(RB, P, COLS)
        in_maps.append({"x": shard, **aux})

    res = run_bass_kernel_spmd(nc, in_maps, list(range(N_CORES)))
    out = np.concatenate(
        [res.results[i]["y"].reshape(SHARD_ROWS, COLS)
         for i in range(N_CORES)], axis=0)

    # The clamp cannot engage for max|w| <= CLAMP - CONS_RATE; apply on host
    # in the corner case so the kernel stays exact for arbitrary inputs.
    if np.abs(w).max() > CLAMP - CONS_RATE:
        np.clip(out, -CLAMP, CLAMP, out=out)
    return out
